# revision 13
# baseline (speedup 1.0000x reference)
"""Trainium2 Bass kernel for nn_CrossFeatureTransformer (V2, transposed-out).

Same folding as V1 (see kernel.py docstring), but scores/vp are produced in
[n, c] orientation per (b,h) pair: the per-bh slice of the [c8, m]-layout h1e
tile doubles as the transposed stationary operand, so no transposes are
needed. The softmax n-reductions (den = sum_n e, num = sum_n e*vp) then
become tensor-engine contractions over partitions: one N=512 matmul per bh
whose lhsT is an all-ones column at position 4i+b (a sliding slice of a
[128,255] band constant) accumulates [den | num] into psum row 4i+b of a
single persistent bank. Normalization happens once at the tail. The Vector
engine only does the e*vp product per chunk.

All matmuls stay in (128,128) tile mode (zero-padded weights, memset-once
finite padding rows). dennum matmuls for chunk i-1 are issued after sc of
chunk i so the PE never waits on exp/mult.
"""

import numpy as np
import ml_dtypes

import concourse.bass as bass
import concourse.bacc as bacc
import concourse.mybir as mybir
from concourse.tile import TileContext
from concourse.bass_utils import run_bass_kernel_spmd

BF16 = mybir.dt.bfloat16
FP8 = mybir.dt.float8e4
F32 = mybir.dt.float32
DR = mybir.MatmulPerfMode.DoubleRow
AX = mybir.AxisListType
ALU = mybir.AluOpType
ACTF = mybir.ActivationFunctionType

B, H, N, C = 16, 64, 128, 256
HID = 1024
C8 = 32
EPS = 1e-6
NCORES = 8
BH = (B // NCORES) * H          # 128 (b,h) pairs per core
M = BH * N                      # 16384 columns per core
CHUNK_BH = 4
CHUNK = CHUNK_BH * N            # 512
NCHUNK = M // CHUNK             # 32
PEN = -10000.0

_cache = {}


def _build_nc():
    nc = bacc.Bacc("TRN2", target_bir_lowering=False, debug=False)

    # ---- DRAM I/O ----
    ktT = nc.dram_tensor("ktT", [128, 2, M], FP8, kind="ExternalInput")
    auxd = nc.dram_tensor("auxd", [66, M], BF16, kind="ExternalInput")
    query = nc.dram_tensor("query", [BH, C], F32, kind="ExternalInput")
    a1p = nc.dram_tensor("a1p", [128, 2, 128], FP8, kind="ExternalInput")
    b1ip = nc.dram_tensor("b1ip", [128, 128], BF16, kind="ExternalInput")
    sc2eT = nc.dram_tensor("sc2eT", [128, C], BF16, kind="ExternalInput")
    wvT = nc.dram_tensor("wvT", [128, 2, C], FP8, kind="ExternalInput")
    posw2T = nc.dram_tensor("posw2T", [128, C], BF16, kind="ExternalInput")
    wo = nc.dram_tensor("wo", [128, 2, C], F32, kind="ExternalInput")
    bor = nc.dram_tensor("bor", [1, C], F32, kind="ExternalInput")
    ff1 = nc.dram_tensor("ff1", [128, 2, HID], BF16, kind="ExternalInput")
    ff2 = nc.dram_tensor("ff2", [128, 8, C], BF16, kind="ExternalInput")
    ffb1p = nc.dram_tensor("ffb1p", [1, 8, 128], BF16, kind="ExternalInput")
    ffb2 = nc.dram_tensor("ffb2", [1, C], F32, kind="ExternalInput")
    out = nc.dram_tensor("out", [BH, C], F32, kind="ExternalOutput")

    NKT, NAUX, NH1E, NEP = 8, 8, 4, 4

    with TileContext(nc) as tc, tc.tile_pool(name="consts", bufs=1) as cpool:
        def T(shape, dtype, name):
            return cpool.tile(shape, dtype, tag=name, name=name)

        # ---- persistent SBUF constants ----
        a1p_sb = T([128, 2, 128], FP8, "a1p_sb")
        nc.scalar.dma_start(a1p_sb, a1p[:])
        b1ip_sb = T([128, 128], BF16, "b1ip_sb")
        nc.scalar.dma_start(b1ip_sb, b1ip[:])
        sc2eT_sb = T([128, C], BF16, "sc2eT_sb")
        nc.scalar.dma_start(sc2eT_sb, sc2eT[:])
        wvT_sb = T([128, 2, C], FP8, "wvT_sb")
        nc.scalar.dma_start(wvT_sb, wvT[:])
        posw2T_sb = T([128, C], BF16, "posw2T_sb")
        nc.scalar.dma_start(posw2T_sb, posw2T[:])
        # tail-only weights: DMA'd from inside the loop (i==1) so they don't
        # delay the first chunks' kt/aux transfers
        wo_sb = T([128, 2, C], F32, "wo_sb")
        ff1_sb = T([128, 2, HID], BF16, "ff1_sb")
        ff2_sb = T([128, 8, C], BF16, "ff2_sb")
        query_sb = T([BH, C], F32, "query_sb")

        # bias rows padded to full-K matmuls: row 0 = data, rows 1:128 = 0
        onespad_sb = T([128, 128], F32, "onespad_sb")
        nc.vector.memset(onespad_sb, 0.0)
        nc.vector.memset(onespad_sb[0:1], 1.0)
        borpad_sb = T([128, C], F32, "borpad_sb")
        nc.vector.memset(borpad_sb, 0.0)
        nc.scalar.dma_start(borpad_sb[0:1], bor[:])
        ffb2pad_sb = T([128, C], F32, "ffb2pad_sb")
        nc.vector.memset(ffb2pad_sb, 0.0)
        nc.scalar.dma_start(ffb2pad_sb[0:1], ffb2[:])
        ffb1p_sb = T([128, 8, 128], BF16, "ffb1p_sb")
        nc.vector.memset(ffb1p_sb, 0.0)
        nc.scalar.dma_start(ffb1p_sb[0:1], ffb1p[:])
        onescols_sb = T([128, 128], BF16, "onescols_sb")
        nc.vector.memset(onescols_sb, 0.0)
        nc.vector.memset(onescols_sb[0:1], 1.0)

        # band[:, 127] = 1, else 0; slice [127-j : 255-j] = ones column j
        band_sb = T([128, 255], BF16, "band_sb")
        nc.vector.memset(band_sb, 0.0)
        nc.vector.memset(band_sb[:, 127:128], 1.0)

        warm_sb = T([1, 8], F32, "warm_sb")
        nc.vector.memset(warm_sb, 0.0)
        nc.scalar.activation(warm_sb, warm_sb, ACTF.Exp)

        ident_sb = T([128, 128], F32, "ident_sb")
        from concourse.masks import make_identity
        make_identity(nc, ident_sb)

        with (
            tc.tile_pool(name="ktp", bufs=NKT) as ktp,
            tc.tile_pool(name="auxp", bufs=NAUX) as auxp,
            tc.tile_pool(name="h1ep", bufs=NH1E) as h1ep,
            tc.tile_pool(name="epp", bufs=NEP) as epp,
            tc.tile_pool(name="ps_h1", bufs=1, space="PSUM") as ps_h1,
            tc.tile_pool(name="ps_sc", bufs=1, space="PSUM") as ps_sc,
            tc.tile_pool(name="ps_vp", bufs=2, space="PSUM") as ps_vp,
            tc.tile_pool(name="ps_dn", bufs=1, space="PSUM") as ps_dn,
        ):
            # persistent [den | num] accumulator rows, one bank
            dnps = ps_dn.tile([128, 2, C], F32, tag="dn", name="dnps")
            pend = []

            def dennum(state):
                ep_p, i_p = state
                for b in range(CHUNK_BH):
                    j = i_p * CHUNK_BH + b
                    nc.tensor.matmul(dnps, band_sb[:, 127 - j:255 - j],
                                     ep_p[:, b, :, :],
                                     start=(j == 0), stop=(j == BH - 1))

            for i in range(NCHUNK):
                cs = slice(i * CHUNK, (i + 1) * CHUNK)

                kt = ktp.tile([128, 2, CHUNK], FP8, tag="kt", name="kt")
                nc.sync.dma_start(kt, ktT[:, :, cs])
                aux = auxp.tile([128, CHUNK], BF16, tag="aux", name="aux")
                if i < NAUX:
                    nc.vector.memset(aux[64:128], 0.0)
                nc.sync.dma_start(aux[0:66], auxd[:, cs])
                h1e = h1ep.tile([128, CHUNK], BF16, tag="h1e", name="h1e")
                if i < NH1E:
                    nc.vector.memset(h1e[32:64], 0.0)
                    nc.vector.memset(h1e[64:128], 0.0)
                if i == 1:
                    nc.scalar.dma_start(wo_sb, wo[:])
                    nc.scalar.dma_start(ff1_sb, ff1[:])
                    nc.scalar.dma_start(ff2_sb, ff2[:])
                    nc.scalar.dma_start(query_sb, query[:])

                # h1 psum: key@A1 + pos_h@B1 + beta; rows 32/33 = (1-m), 1
                h1ps = ps_h1.tile([128, CHUNK], F32, tag="h1", name="h1ps")
                nc.tensor.matmul(h1ps, a1p_sb, kt, start=True,
                                 stop=False, perf_mode=DR)
                nc.tensor.matmul(h1ps, b1ip_sb, aux, start=False, stop=True)
                nc.scalar.activation(h1e[0:34], h1ps[0:34], ACTF.Relu)

                # vp_b [n, c] = key_b@Wv + pos_h_b@posw2  (per bh)
                vpps = ps_vp.tile([128, CHUNK_BH, C], F32, tag="vp",
                                  name="vpps")
                for b in range(CHUNK_BH):
                    bs = slice(b * N, (b + 1) * N)
                    nc.tensor.matmul(vpps[:, b, :], kt[:, :, bs],
                                     wvT_sb, start=True, stop=False,
                                     perf_mode=DR)
                    nc.tensor.matmul(vpps[:, b, :], aux[:, bs],
                                     posw2T_sb, start=False, stop=True)

                # scores_b [n, c] = h1e_b.T @ [sc_w2; -1e4; sc_b2]
                scps = ps_sc.tile([128, CHUNK_BH, C], F32, tag="sc",
                                  name="scps")
                for b in range(CHUNK_BH):
                    bs = slice(b * N, (b + 1) * N)
                    nc.tensor.matmul(scps[:, b, :], h1e[:, bs], sc2eT_sb,
                                     start=True, stop=True)

                # den/num matmuls lag two chunks so their exp/mult
                # inputs are always ready when the PE reaches them
                if len(pend) == 2:
                    dennum(pend.pop(0))

                # e = exp(scores); prod = e*vp
                ep = epp.tile([128, CHUNK_BH, 2, C], BF16, tag="ep",
                              name="ep")
                nc.scalar.activation(ep[:, :, 0, :], scps, ACTF.Exp)
                nc.vector.tensor_tensor(ep[:, :, 1, :], ep[:, :, 0, :],
                                        vpps, ALU.mult)
                pend.append((ep, i))

            for state in pend:
                dennum(state)

            # ---- tail: normalize, transpose agg, attn_out, LN2, FF ----
            rec_sb = T([BH, C], F32, "rec_sb")
            nc.vector.reciprocal(rec_sb, dnps[:, 0, :])
            agg2_sb = T([BH, C], F32, "agg2_sb")
            nc.vector.tensor_tensor(agg2_sb, dnps[:, 1, :], rec_sb, ALU.mult)

            aggT_sb = T([128, 2, BH], F32, "aggT_sb")
            for ct in range(2):
                tp_ps = ps_h1.tile([128, 128], F32, tag="h1", name="tp_ps")
                nc.tensor.transpose(tp_ps,
                                    agg2_sb[:, ct * 128:(ct + 1) * 128],
                                    ident_sb)
                nc.vector.tensor_copy(aggT_sb[:, ct, :], tp_ps)

            at_ps = ps_sc.tile([BH, C], F32, tag="sc", name="at_ps")
            nc.tensor.matmul(at_ps, aggT_sb[:, 0, :], wo_sb[:, 0, :],
                             start=True, stop=False)
            nc.tensor.matmul(at_ps, aggT_sb[:, 1, :], wo_sb[:, 1, :],
                             start=False, stop=False)
            nc.tensor.matmul(at_ps, onespad_sb, borpad_sb,
                             start=False, stop=True)
            x2_sb = T([BH, C], F32, "x2_sb")
            nc.vector.tensor_tensor(x2_sb, at_ps, query_sb, ALU.add)

            # LN2 (affine folded into ff_w1/ff_b1 on host)
            scol = T([BH, 1], F32, "scol")
            nc.vector.tensor_reduce(scol, x2_sb, axis=AX.X, op=ALU.add)
            mcol = T([BH, 1], F32, "mcol")
            nc.vector.tensor_scalar_mul(mcol, scol, 1.0 / C)
            xc_sb = T([BH, C], F32, "xc_sb")
            nc.vector.tensor_scalar(xc_sb, x2_sb, mcol, None,
                                    op0=ALU.subtract)
            sq_sb = T([BH, C], F32, "sq_sb")
            ss_col = T([BH, 1], F32, "ss_col")
            nc.scalar.activation(sq_sb, xc_sb, ACTF.Square, accum_out=ss_col)
            std_col = T([BH, 1], F32, "std_col")
            eps_col = T([BH, 1], F32, "eps_col")
            nc.vector.memset(eps_col, EPS)
            nc.scalar.activation(std_col, ss_col, ACTF.Sqrt,
                                 bias=eps_col, scale=1.0 / C)
            rstd_col = T([BH, 1], F32, "rstd_col")
            nc.vector.reciprocal(rstd_col, std_col)
            y0_sb = T([BH, C], F32, "y0_sb")
            nc.vector.tensor_scalar(y0_sb, xc_sb, rstd_col, None,
                                    op0=ALU.mult)

            # y0T (bf16) via PE transpose
            y0t_sb = T([128, 2, BH], BF16, "y0t_sb")
            for ct in range(2):
                tp_ps = ps_h1.tile([128, 128], F32, tag="h1", name="tp_ps")
                nc.tensor.transpose(tp_ps,
                                    y0_sb[:, ct * 128:(ct + 1) * 128],
                                    ident_sb)
                nc.vector.tensor_copy(y0t_sb[:, ct, :], tp_ps)

            # FF: hidden = relu(y0@ff1 + ffb1), out = hidden@ff2 + ffb2
            ht_sb = T([128, 8, BH], BF16, "ht_sb")
            for hw in range(2):
                ff_ps = ps_vp.tile([128, 4, BH], F32, tag="vp", name="ff_ps")
                for hq in range(4):
                    ht = hw * 4 + hq
                    hsl = slice(ht * 128, (ht + 1) * 128)
                    nc.tensor.matmul(ff_ps[:, hq, :], ff1_sb[:, 0, hsl],
                                     y0t_sb[:, 0, :], start=True, stop=False)
                    nc.tensor.matmul(ff_ps[:, hq, :], ff1_sb[:, 1, hsl],
                                     y0t_sb[:, 1, :], start=False,
                                     stop=False)
                    nc.tensor.matmul(ff_ps[:, hq, :], ffb1p_sb[:, ht, :],
                                     onescols_sb, start=False, stop=True)
                nc.scalar.activation(
                    ht_sb[:, hw * 4:(hw + 1) * 4, :], ff_ps, ACTF.Relu)
            y_ps = ps_sc.tile([BH, C], F32, tag="sc", name="y_ps")
            for ht in range(8):
                nc.tensor.matmul(y_ps, ht_sb[:, ht, :], ff2_sb[:, ht, :],
                                 start=(ht == 0), stop=False)
            nc.tensor.matmul(y_ps, onespad_sb, ffb2pad_sb,
                             start=False, stop=True)
            out_sb = T([BH, C], F32, "out_sb")
            nc.vector.tensor_tensor(out_sb, y_ps, x2_sb, ALU.add)
            nc.sync.dma_start(out[:], out_sb)

    nc.compile()
    return nc


def _ln_np(x, g, b):
    m = x.mean(-1, keepdims=True)
    v = ((x - m) ** 2).mean(-1, keepdims=True)
    return (x - m) / np.sqrt(v + EPS) * g + b


def _prep(inputs):
    f = {k: np.asarray(v, np.float64) for k, v in inputs.items()
         if k != "visibility_mask"}
    mask = np.asarray(inputs["visibility_mask"])
    bf = ml_dtypes.bfloat16
    f8 = ml_dtypes.float8_e4m3

    A1 = f["Wk"] @ f["sc_w1"]                       # [C, 32]
    B1 = f["pos_w2"] @ f["sc_w1"]                   # [32, 32]
    c1 = f["pos_b2"] @ f["sc_w1"] + f["sc_b1"]      # [32]
    q = _ln_np(f["query_input"], f["ln1_g"], f["ln1_b"]) @ f["Wq"]  # [B,H,C]
    beta = (c1[None, None] - q @ f["sc_w1"]).astype(np.float32)  # [B,H,32]
    bo2 = f["pos_b2"] @ f["Wo"] + f["bo"]           # [C]
    ff1f = np.diag(f["ln2_g"]) @ f["ff_w1"]         # [C, HID]
    ffb1 = f["ln2_b"] @ f["ff_w1"] + f["ff_b1"]     # [HID]

    a1p = np.zeros((256, 128), np.float64)
    a1p[:, 0:C8] = A1
    a1p = np.ascontiguousarray(
        a1p.reshape(2, 128, 128).transpose(1, 0, 2)).astype(f8)

    b1ip = np.zeros((128, 128), np.float64)
    b1ip[0:C8, 0:C8] = B1
    b1ip[C8:2 * C8, 0:C8] = np.eye(C8)
    b1ip[64, 32] = 1.0                               # (1-m) passthrough row
    b1ip[65, 33] = 1.0                               # const-1 row
    b1ip = b1ip.astype(bf)

    sc2eT = np.zeros((128, C), np.float64)
    sc2eT[0:C8] = f["sc_w2"]
    sc2eT[32] = PEN
    sc2eT[33] = f["sc_b2"]
    sc2eT = sc2eT.astype(bf)

    wvT = np.ascontiguousarray(
        f["Wv"].reshape(2, 128, C).transpose(1, 0, 2)).astype(f8)

    posw2T = np.zeros((128, C), np.float64)
    posw2T[0:C8] = f["pos_w2"]
    posw2T = posw2T.astype(bf)

    shared = {
        "a1p": a1p, "b1ip": b1ip, "sc2eT": sc2eT, "wvT": wvT,
        "posw2T": posw2T,
        "wo": np.ascontiguousarray(
            f["Wo"].reshape(2, 128, C).transpose(1, 0, 2)).astype(np.float32),
        "bor": bo2.reshape(1, C).astype(np.float32),
        "ff1": np.ascontiguousarray(
            ff1f.reshape(2, 128, HID).transpose(1, 0, 2)).astype(bf),
        "ff2": np.ascontiguousarray(
            f["ff_w2"].reshape(8, 128, C).transpose(1, 0, 2)).astype(bf),
        "ffb1p": ffb1.reshape(1, 8, 128).astype(bf),
        "ffb2": f["ff_b2"].reshape(1, C).astype(np.float32),
    }

    key = np.asarray(inputs["key_input"], np.float32)    # [B,H,N,C]
    quer = np.asarray(inputs["query_input"], np.float32)  # [B,H,C]
    rpos = np.asarray(inputs["relative_pos"], np.float32)  # [B,H,N,4]
    pos_h = np.maximum(
        rpos @ f["pos_w1"].astype(np.float32)
        + f["pos_b1"].astype(np.float32), 0.0)           # [B,H,N,32]
    inv_mask = (mask[..., 0] == 0).astype(np.float32)    # [B,H,N]

    in_maps = []
    bpc = B // NCORES
    for c in range(NCORES):
        bs = slice(c * bpc, (c + 1) * bpc)
        m_ = {}
        ktc = key[bs].reshape(M, C).T                    # [C, M]
        m_["ktT"] = np.ascontiguousarray(
            ktc.reshape(2, 128, M).transpose(1, 0, 2)).astype(f8)
        aux = np.empty((66, M), np.float32)
        aux[0:32] = pos_h[bs].reshape(M, C8).T
        aux[32:64] = np.repeat(beta[bs].reshape(BH, C8), N, axis=0).T
        aux[64] = inv_mask[bs].reshape(M)
        aux[65] = 1.0
        m_["auxd"] = aux.astype(bf)
        m_["query"] = quer[bs].reshape(BH, C).astype(np.float32)
        m_.update(shared)
        in_maps.append(m_)
    return in_maps


def kernel(**inputs):
    if "nc" not in _cache:
        _cache["nc"] = _build_nc()
    nc = _cache["nc"]
    in_maps = _prep(inputs)
    res = run_bass_kernel_spmd(nc, in_maps, core_ids=list(range(NCORES)))
    outs = [r["out"].reshape(B // NCORES, H, C) for r in res.results]
    return np.concatenate(outs, axis=0).astype(np.float32)


# revision 14
# speedup vs baseline: 1.0121x; 1.0121x over previous
"""Trainium2 Bass kernel for nn_CrossFeatureTransformer (V2, transposed-out).

Same folding as V1 (see kernel.py docstring), but scores/vp are produced in
[n, c] orientation per (b,h) pair: the per-bh slice of the [c8, m]-layout h1e
tile doubles as the transposed stationary operand, so no transposes are
needed. The softmax n-reductions (den = sum_n e, num = sum_n e*vp) then
become tensor-engine contractions over partitions: one N=512 matmul per bh
whose lhsT is an all-ones column at position 4i+b (a sliding slice of a
[128,255] band constant) accumulates [den | num] into psum row 4i+b of a
single persistent bank. Normalization happens once at the tail. The Vector
engine only does the e*vp product per chunk.

All matmuls stay in (128,128) tile mode (zero-padded weights, memset-once
finite padding rows). dennum matmuls for chunk i-1 are issued after sc of
chunk i so the PE never waits on exp/mult.
"""

import numpy as np
import ml_dtypes

import concourse.bass as bass
import concourse.bacc as bacc
import concourse.mybir as mybir
from concourse.tile import TileContext
from concourse.bass_utils import run_bass_kernel_spmd

BF16 = mybir.dt.bfloat16
FP8 = mybir.dt.float8e4
F32 = mybir.dt.float32
DR = mybir.MatmulPerfMode.DoubleRow
AX = mybir.AxisListType
ALU = mybir.AluOpType
ACTF = mybir.ActivationFunctionType

B, H, N, C = 16, 64, 128, 256
HID = 1024
C8 = 32
EPS = 1e-6
NCORES = 8
BH = (B // NCORES) * H          # 128 (b,h) pairs per core
M = BH * N                      # 16384 columns per core
CHUNK_BH = 4
CHUNK = CHUNK_BH * N            # 512
NCHUNK = M // CHUNK             # 32
PEN = -10000.0

_cache = {}


def _build_nc():
    nc = bacc.Bacc("TRN2", target_bir_lowering=False, debug=False)

    # ---- DRAM I/O ----
    ktT = nc.dram_tensor("ktT", [128, 2, M], FP8, kind="ExternalInput")
    auxd = nc.dram_tensor("auxd", [66, M], BF16, kind="ExternalInput")
    query = nc.dram_tensor("query", [BH, C], F32, kind="ExternalInput")
    a1p = nc.dram_tensor("a1p", [128, 2, 128], FP8, kind="ExternalInput")
    b1ip = nc.dram_tensor("b1ip", [128, 128], BF16, kind="ExternalInput")
    sc2eT = nc.dram_tensor("sc2eT", [128, C], BF16, kind="ExternalInput")
    wvT = nc.dram_tensor("wvT", [128, 2, C], FP8, kind="ExternalInput")
    posw2T = nc.dram_tensor("posw2T", [128, C], BF16, kind="ExternalInput")
    wo = nc.dram_tensor("wo", [128, 2, C], BF16, kind="ExternalInput")
    bor = nc.dram_tensor("bor", [1, C], F32, kind="ExternalInput")
    ff1 = nc.dram_tensor("ff1", [128, 2, HID], BF16, kind="ExternalInput")
    ff2 = nc.dram_tensor("ff2", [128, 8, C], BF16, kind="ExternalInput")
    ffb1p = nc.dram_tensor("ffb1p", [1, 8, 128], BF16, kind="ExternalInput")
    ffb2 = nc.dram_tensor("ffb2", [1, C], F32, kind="ExternalInput")
    out = nc.dram_tensor("out", [BH, C], F32, kind="ExternalOutput")

    NKT, NAUX, NH1E, NEP = 8, 8, 4, 4

    with TileContext(nc) as tc, tc.tile_pool(name="consts", bufs=1) as cpool:
        def T(shape, dtype, name):
            return cpool.tile(shape, dtype, tag=name, name=name)

        # ---- persistent SBUF constants ----
        a1p_sb = T([128, 2, 128], FP8, "a1p_sb")
        nc.scalar.dma_start(a1p_sb, a1p[:])
        b1ip_sb = T([128, 128], BF16, "b1ip_sb")
        nc.scalar.dma_start(b1ip_sb, b1ip[:])
        sc2eT_sb = T([128, C], BF16, "sc2eT_sb")
        nc.scalar.dma_start(sc2eT_sb, sc2eT[:])
        wvT_sb = T([128, 2, C], FP8, "wvT_sb")
        nc.scalar.dma_start(wvT_sb, wvT[:])
        posw2T_sb = T([128, C], BF16, "posw2T_sb")
        nc.scalar.dma_start(posw2T_sb, posw2T[:])
        # tail-only weights: DMA'd from inside the loop (i==1) so they don't
        # delay the first chunks' kt/aux transfers
        wo_sb = T([128, 2, C], BF16, "wo_sb")
        ff1_sb = T([128, 2, HID], BF16, "ff1_sb")
        ff2_sb = T([128, 8, C], BF16, "ff2_sb")
        query_sb = T([BH, C], F32, "query_sb")

        # bias rows padded to full-K matmuls: row 0 = data, rows 1:128 = 0
        # (memsets on GpSimd so the Vector queue starts on loop work; the
        # bias-row DMAs are issued post-loop, they are tail-only)
        onespad_sb = T([128, 128], F32, "onespad_sb")
        nc.gpsimd.memset(onespad_sb, 0.0)
        nc.gpsimd.memset(onespad_sb[0:1], 1.0)
        borpad_sb = T([128, C], F32, "borpad_sb")
        nc.gpsimd.memset(borpad_sb, 0.0)
        ffb2pad_sb = T([128, C], F32, "ffb2pad_sb")
        nc.gpsimd.memset(ffb2pad_sb, 0.0)
        ffb1p_sb = T([128, 8, 128], BF16, "ffb1p_sb")
        nc.gpsimd.memset(ffb1p_sb, 0.0)
        onescols_sb = T([128, 128], BF16, "onescols_sb")
        nc.gpsimd.memset(onescols_sb, 0.0)
        nc.gpsimd.memset(onescols_sb[0:1], 1.0)

        # band[:, 127] = 1, else 0; slice [127-j : 255-j] = ones column j
        band_sb = T([128, 255], BF16, "band_sb")
        nc.vector.memset(band_sb, 0.0)
        nc.vector.memset(band_sb[:, 127:128], 1.0)

        warm_sb = T([1, 8], F32, "warm_sb")
        nc.vector.memset(warm_sb, 0.0)
        nc.scalar.activation(warm_sb, warm_sb, ACTF.Exp)

        ident_sb = T([128, 128], F32, "ident_sb")

        with (
            tc.tile_pool(name="ktp", bufs=NKT) as ktp,
            tc.tile_pool(name="auxp", bufs=NAUX) as auxp,
            tc.tile_pool(name="h1ep", bufs=NH1E) as h1ep,
            tc.tile_pool(name="epp", bufs=NEP) as epp,
            tc.tile_pool(name="ps_h1", bufs=1, space="PSUM") as ps_h1,
            tc.tile_pool(name="ps_sc", bufs=1, space="PSUM") as ps_sc,
            tc.tile_pool(name="ps_vp", bufs=2, space="PSUM") as ps_vp,
            tc.tile_pool(name="ps_dn", bufs=1, space="PSUM") as ps_dn,
        ):
            # persistent [den | num] accumulator rows, one bank
            dnps = ps_dn.tile([128, 2, C], F32, tag="dn", name="dnps")
            rec_sb = T([BH, C], F32, "rec_sb")
            agg2_sb = T([BH, C], BF16, "agg2_sb")
            pend = []

            def normalize(g):
                gs = slice(32 * g, 32 * (g + 1))
                nc.vector.reciprocal(rec_sb[gs], dnps[gs, 0, :])
                nc.vector.tensor_tensor(agg2_sb[gs], dnps[gs, 1, :],
                                        rec_sb[gs], ALU.mult)

            def dennum(state):
                ep_p, i_p = state
                for b in range(CHUNK_BH):
                    j = i_p * CHUNK_BH + b
                    nc.tensor.matmul(dnps, band_sb[:, 127 - j:255 - j],
                                     ep_p[:, b, :, :],
                                     start=(j == 0), stop=(j == BH - 1))

            for i in range(NCHUNK):
                cs = slice(i * CHUNK, (i + 1) * CHUNK)

                kt = ktp.tile([128, 2, CHUNK], FP8, tag="kt", name="kt")
                nc.sync.dma_start(kt, ktT[:, :, cs])
                aux = auxp.tile([128, CHUNK], BF16, tag="aux", name="aux")
                if i < NAUX:
                    nc.vector.memset(aux[64:128], 0.0)
                nc.sync.dma_start(aux[0:66], auxd[:, cs])
                h1e = h1ep.tile([128, CHUNK], BF16, tag="h1e", name="h1e")
                if i < NH1E:
                    nc.vector.memset(h1e[32:64], 0.0)
                    nc.vector.memset(h1e[64:128], 0.0)
                if i == 1:
                    nc.scalar.dma_start(wo_sb, wo[:])
                    nc.scalar.dma_start(ff1_sb, ff1[:])
                    nc.scalar.dma_start(ff2_sb, ff2[:])
                    nc.scalar.dma_start(query_sb, query[:])

                # h1 psum: key@A1 + pos_h@B1 + beta; rows 32/33 = (1-m), 1
                h1ps = ps_h1.tile([128, CHUNK], F32, tag="h1", name="h1ps")
                nc.tensor.matmul(h1ps, a1p_sb, kt, start=True,
                                 stop=False, perf_mode=DR)
                nc.tensor.matmul(h1ps, b1ip_sb, aux, start=False, stop=True)
                nc.scalar.activation(h1e[0:34], h1ps[0:34], ACTF.Relu)

                # vp_b [n, c] = key_b@Wv + pos_h_b@posw2  (per bh)
                vpps = ps_vp.tile([128, CHUNK_BH, C], F32, tag="vp",
                                  name="vpps")
                for b in range(CHUNK_BH):
                    bs = slice(b * N, (b + 1) * N)
                    nc.tensor.matmul(vpps[:, b, :], kt[:, :, bs],
                                     wvT_sb, start=True, stop=False,
                                     perf_mode=DR)
                    nc.tensor.matmul(vpps[:, b, :], aux[:, bs],
                                     posw2T_sb, start=False, stop=True)

                # scores_b [n, c] = h1e_b.T @ [sc_w2; -1e4; sc_b2]
                scps = ps_sc.tile([128, CHUNK_BH, C], F32, tag="sc",
                                  name="scps")
                for b in range(CHUNK_BH):
                    bs = slice(b * N, (b + 1) * N)
                    nc.tensor.matmul(scps[:, b, :], h1e[:, bs], sc2eT_sb,
                                     start=True, stop=True)

                # den/num matmuls lag two chunks so their exp/mult
                # inputs are always ready when the PE reaches them
                if len(pend) == 2:
                    dennum(pend.pop(0))

                # e = exp(scores); prod = e*vp
                ep = epp.tile([128, CHUNK_BH, 2, C], BF16, tag="ep",
                              name="ep")
                nc.scalar.activation(ep[:, :, 0, :], scps, ACTF.Exp)
                nc.vector.tensor_tensor(ep[:, :, 1, :], ep[:, :, 0, :],
                                        vpps, ALU.mult)
                pend.append((ep, i))
                # rows 32g:32g+32 of dnps complete after dennum(8g+7),
                # issued at iteration 8g+9 -> normalize groups 0..2 overlap
                # the loop; group 3 happens after the final dennum below
                if i in (10, 18, 26):
                    normalize((i - 10) // 8)

            for state in pend:
                dennum(state)
            # prefetch the sqrt table while PE/DVE drain the loop
            nc.scalar.activation(warm_sb, warm_sb, ACTF.Sqrt)
            nc.scalar.dma_start(borpad_sb[0:1], bor[:])
            nc.scalar.dma_start(ffb2pad_sb[0:1], ffb2[:])
            nc.scalar.dma_start(ffb1p_sb[0:1], ffb1p[:])
            from concourse.masks import make_identity
            make_identity(nc, ident_sb)
            normalize(3)

            # ---- tail: transpose agg, attn_out, LN2, FF ----
            ident16_sb = T([128, 128], BF16, "ident16_sb")
            nc.vector.tensor_copy(ident16_sb, ident_sb)
            aggT_sb = T([128, 2, BH], BF16, "aggT_sb")
            for ct in range(2):
                tp_ps = ps_h1.tile([128, 128], BF16, tag="h1", name="tp_ps")
                nc.tensor.transpose(tp_ps,
                                    agg2_sb[:, ct * 128:(ct + 1) * 128],
                                    ident16_sb)
                nc.vector.tensor_copy(aggT_sb[:, ct, :], tp_ps)

            at_ps = ps_sc.tile([BH, C], F32, tag="sc", name="at_ps")
            nc.tensor.matmul(at_ps, aggT_sb[:, 0, :], wo_sb[:, 0, :],
                             start=True, stop=False)
            nc.tensor.matmul(at_ps, aggT_sb[:, 1, :], wo_sb[:, 1, :],
                             start=False, stop=False)
            nc.tensor.matmul(at_ps, onespad_sb, borpad_sb,
                             start=False, stop=True)
            x2_sb = T([BH, C], F32, "x2_sb")
            nc.vector.tensor_tensor(x2_sb, at_ps, query_sb, ALU.add)

            # LN2 (affine folded into ff_w1/ff_b1 on host)
            scol = T([BH, 1], F32, "scol")
            nc.vector.tensor_reduce(scol, x2_sb, axis=AX.X, op=ALU.add)
            mcol = T([BH, 1], F32, "mcol")
            nc.vector.tensor_scalar_mul(mcol, scol, 1.0 / C)
            xc_sb = T([BH, C], F32, "xc_sb")
            nc.vector.tensor_scalar(xc_sb, x2_sb, mcol, None,
                                    op0=ALU.subtract)
            sq_sb = T([BH, C], F32, "sq_sb")
            ss_col = T([BH, 1], F32, "ss_col")
            nc.scalar.activation(sq_sb, xc_sb, ACTF.Square, accum_out=ss_col)
            std_col = T([BH, 1], F32, "std_col")
            eps_col = T([BH, 1], F32, "eps_col")
            nc.vector.memset(eps_col, EPS)
            nc.scalar.activation(std_col, ss_col, ACTF.Sqrt,
                                 bias=eps_col, scale=1.0 / C)
            rstd_col = T([BH, 1], F32, "rstd_col")
            nc.vector.reciprocal(rstd_col, std_col)
            y0_sb = T([BH, C], F32, "y0_sb")
            nc.vector.tensor_scalar(y0_sb, xc_sb, rstd_col, None,
                                    op0=ALU.mult)

            # y0T (bf16) via PE transpose
            y0t_sb = T([128, 2, BH], BF16, "y0t_sb")
            for ct in range(2):
                tp_ps = ps_h1.tile([128, 128], F32, tag="h1", name="tp_ps")
                nc.tensor.transpose(tp_ps,
                                    y0_sb[:, ct * 128:(ct + 1) * 128],
                                    ident_sb)
                nc.vector.tensor_copy(y0t_sb[:, ct, :], tp_ps)

            # FF: hidden = relu(y0@ff1 + ffb1), out = hidden@ff2 + ffb2
            ht_sb = T([128, 8, BH], BF16, "ht_sb")
            for hw in range(2):
                ff_ps = ps_vp.tile([128, 4, BH], F32, tag="vp", name="ff_ps")
                for hq in range(4):
                    ht = hw * 4 + hq
                    hsl = slice(ht * 128, (ht + 1) * 128)
                    nc.tensor.matmul(ff_ps[:, hq, :], ff1_sb[:, 0, hsl],
                                     y0t_sb[:, 0, :], start=True, stop=False)
                    nc.tensor.matmul(ff_ps[:, hq, :], ff1_sb[:, 1, hsl],
                                     y0t_sb[:, 1, :], start=False,
                                     stop=False)
                    nc.tensor.matmul(ff_ps[:, hq, :], ffb1p_sb[:, ht, :],
                                     onescols_sb, start=False, stop=True)
                nc.scalar.activation(
                    ht_sb[:, hw * 4:(hw + 1) * 4, :], ff_ps, ACTF.Relu)
            y_ps = ps_sc.tile([BH, C], F32, tag="sc", name="y_ps")
            for ht in range(8):
                nc.tensor.matmul(y_ps, ht_sb[:, ht, :], ff2_sb[:, ht, :],
                                 start=(ht == 0), stop=False)
            nc.tensor.matmul(y_ps, onespad_sb, ffb2pad_sb,
                             start=False, stop=True)
            out_sb = T([BH, C], F32, "out_sb")
            nc.vector.tensor_tensor(out_sb, y_ps, x2_sb, ALU.add)
            nc.sync.dma_start(out[:], out_sb)

    nc.compile()
    return nc


def _ln_np(x, g, b):
    m = x.mean(-1, keepdims=True)
    v = ((x - m) ** 2).mean(-1, keepdims=True)
    return (x - m) / np.sqrt(v + EPS) * g + b


def _prep(inputs):
    f = {k: np.asarray(v, np.float64) for k, v in inputs.items()
         if k != "visibility_mask"}
    mask = np.asarray(inputs["visibility_mask"])
    bf = ml_dtypes.bfloat16
    f8 = ml_dtypes.float8_e4m3

    A1 = f["Wk"] @ f["sc_w1"]                       # [C, 32]
    B1 = f["pos_w2"] @ f["sc_w1"]                   # [32, 32]
    c1 = f["pos_b2"] @ f["sc_w1"] + f["sc_b1"]      # [32]
    q = _ln_np(f["query_input"], f["ln1_g"], f["ln1_b"]) @ f["Wq"]  # [B,H,C]
    beta = (c1[None, None] - q @ f["sc_w1"]).astype(np.float32)  # [B,H,32]
    bo2 = f["pos_b2"] @ f["Wo"] + f["bo"]           # [C]
    ff1f = np.diag(f["ln2_g"]) @ f["ff_w1"]         # [C, HID]
    ffb1 = f["ln2_b"] @ f["ff_w1"] + f["ff_b1"]     # [HID]

    a1p = np.zeros((256, 128), np.float64)
    a1p[:, 0:C8] = A1
    a1p = np.ascontiguousarray(
        a1p.reshape(2, 128, 128).transpose(1, 0, 2)).astype(f8)

    b1ip = np.zeros((128, 128), np.float64)
    b1ip[0:C8, 0:C8] = B1
    b1ip[C8:2 * C8, 0:C8] = np.eye(C8)
    b1ip[64, 32] = 1.0                               # (1-m) passthrough row
    b1ip[65, 33] = 1.0                               # const-1 row
    b1ip = b1ip.astype(bf)

    sc2eT = np.zeros((128, C), np.float64)
    sc2eT[0:C8] = f["sc_w2"]
    sc2eT[32] = PEN
    sc2eT[33] = f["sc_b2"]
    sc2eT = sc2eT.astype(bf)

    wvT = np.ascontiguousarray(
        f["Wv"].reshape(2, 128, C).transpose(1, 0, 2)).astype(f8)

    posw2T = np.zeros((128, C), np.float64)
    posw2T[0:C8] = f["pos_w2"]
    posw2T = posw2T.astype(bf)

    shared = {
        "a1p": a1p, "b1ip": b1ip, "sc2eT": sc2eT, "wvT": wvT,
        "posw2T": posw2T,
        "wo": np.ascontiguousarray(
            f["Wo"].reshape(2, 128, C).transpose(1, 0, 2)).astype(bf),
        "bor": bo2.reshape(1, C).astype(np.float32),
        "ff1": np.ascontiguousarray(
            ff1f.reshape(2, 128, HID).transpose(1, 0, 2)).astype(bf),
        "ff2": np.ascontiguousarray(
            f["ff_w2"].reshape(8, 128, C).transpose(1, 0, 2)).astype(bf),
        "ffb1p": ffb1.reshape(1, 8, 128).astype(bf),
        "ffb2": f["ff_b2"].reshape(1, C).astype(np.float32),
    }

    key = np.asarray(inputs["key_input"], np.float32)    # [B,H,N,C]
    quer = np.asarray(inputs["query_input"], np.float32)  # [B,H,C]
    rpos = np.asarray(inputs["relative_pos"], np.float32)  # [B,H,N,4]
    pos_h = np.maximum(
        rpos @ f["pos_w1"].astype(np.float32)
        + f["pos_b1"].astype(np.float32), 0.0)           # [B,H,N,32]
    inv_mask = (mask[..., 0] == 0).astype(np.float32)    # [B,H,N]

    in_maps = []
    bpc = B // NCORES
    for c in range(NCORES):
        bs = slice(c * bpc, (c + 1) * bpc)
        m_ = {}
        ktc = key[bs].reshape(M, C).T                    # [C, M]
        m_["ktT"] = np.ascontiguousarray(
            ktc.reshape(2, 128, M).transpose(1, 0, 2)).astype(f8)
        aux = np.empty((66, M), np.float32)
        aux[0:32] = pos_h[bs].reshape(M, C8).T
        aux[32:64] = np.repeat(beta[bs].reshape(BH, C8), N, axis=0).T
        aux[64] = inv_mask[bs].reshape(M)
        aux[65] = 1.0
        m_["auxd"] = aux.astype(bf)
        m_["query"] = quer[bs].reshape(BH, C).astype(np.float32)
        m_.update(shared)
        in_maps.append(m_)
    return in_maps


def kernel(**inputs):
    if "nc" not in _cache:
        _cache["nc"] = _build_nc()
    nc = _cache["nc"]
    in_maps = _prep(inputs)
    res = run_bass_kernel_spmd(nc, in_maps, core_ids=list(range(NCORES)))
    outs = [r["out"].reshape(B // NCORES, H, C) for r in res.results]
    return np.concatenate(outs, axis=0).astype(np.float32)


# revision 16
# speedup vs baseline: 1.0214x; 1.0092x over previous
"""Trainium2 Bass kernel for nn_CrossFeatureTransformer (V2, transposed-out).

Same folding as V1 (see kernel.py docstring), but scores/vp are produced in
[n, c] orientation per (b,h) pair: the per-bh slice of the [c8, m]-layout h1e
tile doubles as the transposed stationary operand, so no transposes are
needed. The softmax n-reductions (den = sum_n e, num = sum_n e*vp) then
become tensor-engine contractions over partitions: one N=512 matmul per bh
whose lhsT is an all-ones column at position 4i+b (a sliding slice of a
[128,255] band constant) accumulates [den | num] into psum row 4i+b of a
single persistent bank. Normalization happens once at the tail. The Vector
engine only does the e*vp product per chunk.

All matmuls stay in (128,128) tile mode (zero-padded weights, memset-once
finite padding rows). dennum matmuls for chunk i-1 are issued after sc of
chunk i so the PE never waits on exp/mult.
"""

import numpy as np
import ml_dtypes

import concourse.bass as bass
import concourse.bacc as bacc
import concourse.mybir as mybir
from concourse.tile import TileContext
from concourse.bass_utils import run_bass_kernel_spmd

BF16 = mybir.dt.bfloat16
FP8 = mybir.dt.float8e4
F32 = mybir.dt.float32
DR = mybir.MatmulPerfMode.DoubleRow
AX = mybir.AxisListType
ALU = mybir.AluOpType
ACTF = mybir.ActivationFunctionType

B, H, N, C = 16, 64, 128, 256
HID = 1024
C8 = 32
EPS = 1e-6
NCORES = 8
BH = (B // NCORES) * H          # 128 (b,h) pairs per core
M = BH * N                      # 16384 columns per core
CHUNK_BH = 4
CHUNK = CHUNK_BH * N            # 512
NCHUNK = M // CHUNK             # 32
PEN = -10000.0

_cache = {}


def _build_nc():
    nc = bacc.Bacc("TRN2", target_bir_lowering=False, debug=False)

    # ---- DRAM I/O ----
    ktT = nc.dram_tensor("ktT", [128, 2, M], FP8, kind="ExternalInput")
    auxd = nc.dram_tensor("auxd", [66, M], BF16, kind="ExternalInput")
    query = nc.dram_tensor("query", [BH, C], F32, kind="ExternalInput")
    a1p = nc.dram_tensor("a1p", [128, 2, 128], FP8, kind="ExternalInput")
    b1ip = nc.dram_tensor("b1ip", [128, 128], BF16, kind="ExternalInput")
    sc2eT = nc.dram_tensor("sc2eT", [128, C], BF16, kind="ExternalInput")
    wvT = nc.dram_tensor("wvT", [128, 2, C], FP8, kind="ExternalInput")
    posw2T = nc.dram_tensor("posw2T", [128, C], BF16, kind="ExternalInput")
    wo = nc.dram_tensor("wo", [128, 2, C], BF16, kind="ExternalInput")
    bor = nc.dram_tensor("bor", [1, C], BF16, kind="ExternalInput")
    ff1 = nc.dram_tensor("ff1", [128, 2, HID], BF16, kind="ExternalInput")
    ff2 = nc.dram_tensor("ff2", [128, 8, C], BF16, kind="ExternalInput")
    ffb1p = nc.dram_tensor("ffb1p", [1, 8, 128], BF16, kind="ExternalInput")
    ffb2 = nc.dram_tensor("ffb2", [1, C], BF16, kind="ExternalInput")
    out = nc.dram_tensor("out", [BH, C], F32, kind="ExternalOutput")

    NKT, NAUX, NH1E, NEP = 8, 8, 4, 4

    with TileContext(nc) as tc, tc.tile_pool(name="consts", bufs=1) as cpool:
        def T(shape, dtype, name):
            return cpool.tile(shape, dtype, tag=name, name=name)

        # ---- persistent SBUF constants ----
        a1p_sb = T([128, 2, 128], FP8, "a1p_sb")
        nc.sync.dma_start(a1p_sb, a1p[:])
        b1ip_sb = T([128, 128], BF16, "b1ip_sb")
        nc.sync.dma_start(b1ip_sb, b1ip[:])
        sc2eT_sb = T([128, C], BF16, "sc2eT_sb")
        nc.sync.dma_start(sc2eT_sb, sc2eT[:])
        wvT_sb = T([128, 2, C], FP8, "wvT_sb")
        nc.sync.dma_start(wvT_sb, wvT[:])
        posw2T_sb = T([128, C], BF16, "posw2T_sb")
        nc.sync.dma_start(posw2T_sb, posw2T[:])
        # tail-only weights: DMA'd from inside the loop (i==1) so they don't
        # delay the first chunks' kt/aux transfers
        wo_sb = T([128, 2, C], BF16, "wo_sb")
        ff1_sb = T([128, 2, HID], BF16, "ff1_sb")
        ff2_sb = T([128, 8, C], BF16, "ff2_sb")
        query_sb = T([BH, C], F32, "query_sb")

        # bias rows padded to full-K matmuls: row 0 = data, rows 1:128 = 0
        # (memsets on GpSimd so the Vector queue starts on loop work; the
        # bias-row DMAs are issued post-loop, they are tail-only)
        onespad_sb = T([128, 128], BF16, "onespad_sb")
        nc.gpsimd.memset(onespad_sb, 0.0)
        nc.gpsimd.memset(onespad_sb[0:1], 1.0)
        borpad_sb = T([128, C], BF16, "borpad_sb")
        nc.gpsimd.memset(borpad_sb, 0.0)
        ffb2pad_sb = T([128, C], BF16, "ffb2pad_sb")
        nc.gpsimd.memset(ffb2pad_sb, 0.0)
        ffb1p_sb = T([128, 8, 128], BF16, "ffb1p_sb")
        nc.gpsimd.memset(ffb1p_sb, 0.0)
        onescols_sb = T([128, 128], BF16, "onescols_sb")
        nc.gpsimd.memset(onescols_sb, 0.0)
        nc.gpsimd.memset(onescols_sb[0:1], 1.0)

        # band[:, 127] = 1, else 0; slice [127-j : 255-j] = ones column j
        band_sb = T([128, 255], BF16, "band_sb")
        nc.vector.memset(band_sb, 0.0)
        nc.vector.memset(band_sb[:, 127:128], 1.0)

        warm_sb = T([1, 8], F32, "warm_sb")
        nc.vector.memset(warm_sb, 0.0)
        nc.scalar.activation(warm_sb, warm_sb, ACTF.Exp)

        ident_sb = T([128, 128], F32, "ident_sb")

        with (
            tc.tile_pool(name="ktp", bufs=NKT) as ktp,
            tc.tile_pool(name="auxp", bufs=NAUX) as auxp,
            tc.tile_pool(name="h1ep", bufs=NH1E) as h1ep,
            tc.tile_pool(name="epp", bufs=NEP) as epp,
            tc.tile_pool(name="ps_h1", bufs=1, space="PSUM") as ps_h1,
            tc.tile_pool(name="ps_sc", bufs=1, space="PSUM") as ps_sc,
            tc.tile_pool(name="ps_vp", bufs=2, space="PSUM") as ps_vp,
            tc.tile_pool(name="ps_dn", bufs=1, space="PSUM") as ps_dn,
        ):
            # persistent [den | num] accumulator rows, one bank
            dnps = ps_dn.tile([128, 2, C], F32, tag="dn", name="dnps")
            x2_sb = T([BH, C], F32, "x2_sb")
            rec_sb = T([BH, C], F32, "rec_sb")
            agg2_sb = T([BH, C], BF16, "agg2_sb")
            pend = []

            def normalize(g):
                gs = slice(32 * g, 32 * (g + 1))
                nc.vector.reciprocal(rec_sb[gs], dnps[gs, 0, :])
                nc.vector.tensor_tensor(agg2_sb[gs], dnps[gs, 1, :],
                                        rec_sb[gs], ALU.mult)

            def dennum(state):
                ep_p, i_p = state
                for b in range(CHUNK_BH):
                    j = i_p * CHUNK_BH + b
                    nc.tensor.matmul(dnps, band_sb[:, 127 - j:255 - j],
                                     ep_p[:, b, :, :],
                                     start=(j == 0), stop=(j == BH - 1))

            for i in range(NCHUNK):
                cs = slice(i * CHUNK, (i + 1) * CHUNK)

                kt = ktp.tile([128, 2, CHUNK], FP8, tag="kt", name="kt")
                nc.sync.dma_start(kt, ktT[:, :, cs])
                aux = auxp.tile([128, CHUNK], BF16, tag="aux", name="aux")
                if i < NAUX:
                    nc.vector.memset(aux[64:128], 0.0)
                nc.sync.dma_start(aux[0:66], auxd[:, cs])
                h1e = h1ep.tile([128, CHUNK], BF16, tag="h1e", name="h1e")
                if i < NH1E:
                    nc.vector.memset(h1e[32:64], 0.0)
                    nc.vector.memset(h1e[64:128], 0.0)
                if i == 1:
                    nc.scalar.dma_start(wo_sb, wo[:])
                    nc.scalar.dma_start(ff1_sb, ff1[:])
                    nc.scalar.dma_start(ff2_sb, ff2[:])
                    nc.scalar.dma_start(query_sb, query[:])

                # h1 psum: key@A1 + pos_h@B1 + beta; rows 32/33 = (1-m), 1
                h1ps = ps_h1.tile([128, CHUNK], F32, tag="h1", name="h1ps")
                nc.tensor.matmul(h1ps, a1p_sb, kt, start=True,
                                 stop=False, perf_mode=DR)
                nc.tensor.matmul(h1ps, b1ip_sb, aux, start=False, stop=True)
                nc.scalar.activation(h1e[0:34], h1ps[0:34], ACTF.Relu)

                # vp_b [n, c] = key_b@Wv + pos_h_b@posw2  (per bh)
                vpps = ps_vp.tile([128, CHUNK_BH, C], F32, tag="vp",
                                  name="vpps")
                for b in range(CHUNK_BH):
                    bs = slice(b * N, (b + 1) * N)
                    nc.tensor.matmul(vpps[:, b, :], kt[:, :, bs],
                                     wvT_sb, start=True, stop=False,
                                     perf_mode=DR)
                    nc.tensor.matmul(vpps[:, b, :], aux[:, bs],
                                     posw2T_sb, start=False, stop=True)

                # scores_b [n, c] = h1e_b.T @ [sc_w2; -1e4; sc_b2]
                scps = ps_sc.tile([128, CHUNK_BH, C], F32, tag="sc",
                                  name="scps")
                for b in range(CHUNK_BH):
                    bs = slice(b * N, (b + 1) * N)
                    nc.tensor.matmul(scps[:, b, :], h1e[:, bs], sc2eT_sb,
                                     start=True, stop=True)

                # den/num matmuls lag two chunks so their exp/mult
                # inputs are always ready when the PE reaches them
                if len(pend) == 2:
                    dennum(pend.pop(0))

                # e = exp(scores); prod = e*vp
                ep = epp.tile([128, CHUNK_BH, 2, C], BF16, tag="ep",
                              name="ep")
                nc.scalar.activation(ep[:, :, 0, :], scps, ACTF.Exp)
                nc.vector.tensor_tensor(ep[:, :, 1, :], ep[:, :, 0, :],
                                        vpps, ALU.mult)
                pend.append((ep, i))
                # rows 32g:32g+32 of dnps complete after dennum(8g+7),
                # issued at iteration 8g+9 -> normalize groups 0..2 overlap
                # the loop; group 3 happens after the final dennum below
                if i in (10, 18, 26):
                    normalize((i - 10) // 8)

            for state in pend:
                dennum(state)
            # prefetch the sqrt table while PE/DVE drain the loop (write a
            # corner of x2_sb so the op isn't dead-code-eliminated; x2 is
            # fully overwritten later)
            nc.scalar.activation(x2_sb[0:1, 0:8], warm_sb, ACTF.Sqrt)
            nc.scalar.dma_start(borpad_sb[0:1], bor[:])
            nc.scalar.dma_start(ffb2pad_sb[0:1], ffb2[:])
            nc.scalar.dma_start(ffb1p_sb[0:1], ffb1p[:])
            from concourse.masks import make_identity
            make_identity(nc, ident_sb)
            ident16_sb = T([128, 128], BF16, "ident16_sb")
            nc.vector.tensor_copy(ident16_sb, ident_sb)
            normalize(3)

            # ---- tail: transpose agg, attn_out, LN2, FF ----
            aggT_sb = T([128, 2, BH], BF16, "aggT_sb")
            for ct in range(2):
                tp_ps = ps_h1.tile([128, 128], BF16, tag="h1", name="tp_ps")
                nc.tensor.transpose(tp_ps,
                                    agg2_sb[:, ct * 128:(ct + 1) * 128],
                                    ident16_sb)
                nc.vector.tensor_copy(aggT_sb[:, ct, :], tp_ps)

            at_ps = ps_sc.tile([BH, C], F32, tag="sc", name="at_ps")
            nc.tensor.matmul(at_ps, aggT_sb[:, 0, :], wo_sb[:, 0, :],
                             start=True, stop=False)
            nc.tensor.matmul(at_ps, aggT_sb[:, 1, :], wo_sb[:, 1, :],
                             start=False, stop=False)
            nc.tensor.matmul(at_ps, onespad_sb, borpad_sb,
                             start=False, stop=True)
            nc.vector.tensor_tensor(x2_sb, at_ps, query_sb, ALU.add)

            # LN2 (affine folded into ff_w1/ff_b1 on host)
            scol = T([BH, 1], F32, "scol")
            nc.vector.tensor_reduce(scol, x2_sb, axis=AX.X, op=ALU.add)
            mcol = T([BH, 1], F32, "mcol")
            nc.vector.tensor_scalar_mul(mcol, scol, 1.0 / C)
            xc_sb = T([BH, C], F32, "xc_sb")
            nc.vector.tensor_scalar(xc_sb, x2_sb, mcol, None,
                                    op0=ALU.subtract)
            sq_sb = T([BH, C], F32, "sq_sb")
            ss_col = T([BH, 1], F32, "ss_col")
            nc.scalar.activation(sq_sb, xc_sb, ACTF.Square, accum_out=ss_col)
            eps_col = T([BH, 1], F32, "eps_col")
            nc.gpsimd.memset(eps_col, EPS)
            std_col = T([BH, 1], F32, "std_col")
            nc.scalar.activation(std_col, ss_col, ACTF.Sqrt,
                                 bias=eps_col, scale=1.0 / C)
            rstd_col = T([BH, 1], F32, "rstd_col")
            nc.vector.reciprocal(rstd_col, std_col)
            y0_sb = T([BH, C], BF16, "y0_sb")
            nc.vector.tensor_scalar(y0_sb, xc_sb, rstd_col, None,
                                    op0=ALU.mult)

            # y0T (bf16) via PE transpose
            y0t_sb = T([128, 2, BH], BF16, "y0t_sb")
            for ct in range(2):
                tp_ps = ps_h1.tile([128, 128], BF16, tag="h1", name="tp_ps")
                nc.tensor.transpose(tp_ps,
                                    y0_sb[:, ct * 128:(ct + 1) * 128],
                                    ident16_sb)
                nc.vector.tensor_copy(y0t_sb[:, ct, :], tp_ps)

            # FF: hidden = relu(y0@ff1 + ffb1), out = hidden@ff2 + ffb2
            ht_sb = T([128, 8, BH], BF16, "ht_sb")
            for hw in range(2):
                ff_ps = ps_vp.tile([128, 4, BH], F32, tag="vp", name="ff_ps")
                for hq in range(4):
                    ht = hw * 4 + hq
                    hsl = slice(ht * 128, (ht + 1) * 128)
                    nc.tensor.matmul(ff_ps[:, hq, :], ff1_sb[:, 0, hsl],
                                     y0t_sb[:, 0, :], start=True, stop=False)
                    nc.tensor.matmul(ff_ps[:, hq, :], ff1_sb[:, 1, hsl],
                                     y0t_sb[:, 1, :], start=False,
                                     stop=False)
                    nc.tensor.matmul(ff_ps[:, hq, :], ffb1p_sb[:, ht, :],
                                     onescols_sb, start=False, stop=True)
                nc.scalar.activation(
                    ht_sb[:, hw * 4:(hw + 1) * 4, :], ff_ps, ACTF.Relu)
            y_ps = ps_sc.tile([BH, C], F32, tag="sc", name="y_ps")
            for ht in range(8):
                nc.tensor.matmul(y_ps, ht_sb[:, ht, :], ff2_sb[:, ht, :],
                                 start=(ht == 0), stop=False)
            nc.tensor.matmul(y_ps, onespad_sb, ffb2pad_sb,
                             start=False, stop=True)
            out_sb = T([BH, C], F32, "out_sb")
            nc.vector.tensor_tensor(out_sb, y_ps, x2_sb, ALU.add)
            nc.sync.dma_start(out[:], out_sb)

    nc.compile()
    return nc


def _ln_np(x, g, b):
    m = x.mean(-1, keepdims=True)
    v = ((x - m) ** 2).mean(-1, keepdims=True)
    return (x - m) / np.sqrt(v + EPS) * g + b


def _prep(inputs):
    f = {k: np.asarray(v, np.float64) for k, v in inputs.items()
         if k != "visibility_mask"}
    mask = np.asarray(inputs["visibility_mask"])
    bf = ml_dtypes.bfloat16
    f8 = ml_dtypes.float8_e4m3

    A1 = f["Wk"] @ f["sc_w1"]                       # [C, 32]
    B1 = f["pos_w2"] @ f["sc_w1"]                   # [32, 32]
    c1 = f["pos_b2"] @ f["sc_w1"] + f["sc_b1"]      # [32]
    q = _ln_np(f["query_input"], f["ln1_g"], f["ln1_b"]) @ f["Wq"]  # [B,H,C]
    beta = (c1[None, None] - q @ f["sc_w1"]).astype(np.float32)  # [B,H,32]
    bo2 = f["pos_b2"] @ f["Wo"] + f["bo"]           # [C]
    ff1f = np.diag(f["ln2_g"]) @ f["ff_w1"]         # [C, HID]
    ffb1 = f["ln2_b"] @ f["ff_w1"] + f["ff_b1"]     # [HID]

    a1p = np.zeros((256, 128), np.float64)
    a1p[:, 0:C8] = A1
    a1p = np.ascontiguousarray(
        a1p.reshape(2, 128, 128).transpose(1, 0, 2)).astype(f8)

    b1ip = np.zeros((128, 128), np.float64)
    b1ip[0:C8, 0:C8] = B1
    b1ip[C8:2 * C8, 0:C8] = np.eye(C8)
    b1ip[64, 32] = 1.0                               # (1-m) passthrough row
    b1ip[65, 33] = 1.0                               # const-1 row
    b1ip = b1ip.astype(bf)

    sc2eT = np.zeros((128, C), np.float64)
    sc2eT[0:C8] = f["sc_w2"]
    sc2eT[32] = PEN
    sc2eT[33] = f["sc_b2"]
    sc2eT = sc2eT.astype(bf)

    wvT = np.ascontiguousarray(
        f["Wv"].reshape(2, 128, C).transpose(1, 0, 2)).astype(f8)

    posw2T = np.zeros((128, C), np.float64)
    posw2T[0:C8] = f["pos_w2"]
    posw2T = posw2T.astype(bf)

    shared = {
        "a1p": a1p, "b1ip": b1ip, "sc2eT": sc2eT, "wvT": wvT,
        "posw2T": posw2T,
        "wo": np.ascontiguousarray(
            f["Wo"].reshape(2, 128, C).transpose(1, 0, 2)).astype(bf),
        "bor": bo2.reshape(1, C).astype(bf),
        "ff1": np.ascontiguousarray(
            ff1f.reshape(2, 128, HID).transpose(1, 0, 2)).astype(bf),
        "ff2": np.ascontiguousarray(
            f["ff_w2"].reshape(8, 128, C).transpose(1, 0, 2)).astype(bf),
        "ffb1p": ffb1.reshape(1, 8, 128).astype(bf),
        "ffb2": f["ff_b2"].reshape(1, C).astype(bf),
    }

    key = np.asarray(inputs["key_input"], np.float32)    # [B,H,N,C]
    quer = np.asarray(inputs["query_input"], np.float32)  # [B,H,C]
    rpos = np.asarray(inputs["relative_pos"], np.float32)  # [B,H,N,4]
    pos_h = np.maximum(
        rpos @ f["pos_w1"].astype(np.float32)
        + f["pos_b1"].astype(np.float32), 0.0)           # [B,H,N,32]
    inv_mask = (mask[..., 0] == 0).astype(np.float32)    # [B,H,N]

    in_maps = []
    bpc = B // NCORES
    for c in range(NCORES):
        bs = slice(c * bpc, (c + 1) * bpc)
        m_ = {}
        ktc = key[bs].reshape(M, C).T                    # [C, M]
        m_["ktT"] = np.ascontiguousarray(
            ktc.reshape(2, 128, M).transpose(1, 0, 2)).astype(f8)
        aux = np.empty((66, M), np.float32)
        aux[0:32] = pos_h[bs].reshape(M, C8).T
        aux[32:64] = np.repeat(beta[bs].reshape(BH, C8), N, axis=0).T
        aux[64] = inv_mask[bs].reshape(M)
        aux[65] = 1.0
        m_["auxd"] = aux.astype(bf)
        m_["query"] = quer[bs].reshape(BH, C).astype(np.float32)
        m_.update(shared)
        in_maps.append(m_)
    return in_maps


def kernel(**inputs):
    if "nc" not in _cache:
        _cache["nc"] = _build_nc()
    nc = _cache["nc"]
    in_maps = _prep(inputs)
    res = run_bass_kernel_spmd(nc, in_maps, core_ids=list(range(NCORES)))
    outs = [r["out"].reshape(B // NCORES, H, C) for r in res.results]
    return np.concatenate(outs, axis=0).astype(np.float32)
